# revision 2
# baseline (speedup 1.0000x reference)
"""Grouped GEMM (MoE expert-parallel) Bass kernel for Trainium2.

Problem: x (16384, 2048) fp32, weight (128*2048, 1408) fp32, batch_sizes (128,)
int32 summing to 16384 (tokens sorted by expert).
out[rows_e] = x[rows_e] @ W[e] for each expert e.

Strategy (expert-parallel across 8 NeuronCores):
  - 16 experts per core. Experts are sorted by batch size (descending) and
    dealt round-robin into 16 "slots" x 8 cores, so slot j holds experts of
    similar size on every core. Slot j gets a static token capacity
    cap_j*128 = max over cores of ceil(bs/128)*128, making the compiled
    program identical (SPMD) across cores.
  - Host pre-transposes/pads x to xT (2048, T_pad) bf16 per core, reorders
    weights to (16, 2048, 1408) bf16 per core. On-chip: out = xT.T @ w via
    TensorE with K=2048 contracted in 16 chunks of 128 accumulating in PSUM.
  - Output (T_pad, 1408) fp32 per core; host strips padding and concatenates.

This file is self-contained: it only needs numpy/ml_dtypes + the concourse
package (importable in the runtime environment).
"""

import os

import numpy as np
import ml_dtypes

import concourse.bass as bass
import concourse.mybir as mybir
import concourse.tile as tile
from concourse import bacc
from concourse.bass_utils import run_bass_kernel_spmd

E = 128          # num experts
M = 2048         # in features (contraction)
N = 1408         # out features
S = 16384        # tokens
NCORES = 8
EPC = E // NCORES      # experts per core = 16
KT = M // 128          # contraction tiles = 16
N_CHUNKS = [(0, 512), (512, 512), (1024, 384)]  # psum-bank-sized N tiles

BF16 = mybir.dt.bfloat16
FP32 = mybir.dt.float32

_program_cache: dict = {}
LAST_EXEC_NS = None
LAST_RESULTS = None


def _build_program(slot_caps):
    """Compile the SPMD Bass program for the given per-slot m-tile counts."""
    T_pad = 128 * int(sum(slot_caps))
    nc = bacc.Bacc(
        "TRN2", target_bir_lowering=False, debug=False, num_devices=NCORES
    )
    xt_d = nc.dram_tensor("xt", [M, T_pad], BF16, kind="ExternalInput").ap()
    w_d = nc.dram_tensor("w", [EPC, M, N], BF16, kind="ExternalInput").ap()
    out_d = nc.dram_tensor("out", [T_pad, N], FP32, kind="ExternalOutput").ap()

    with tile.TileContext(nc) as tc:
        with (
            tc.tile_pool(name="wp", bufs=3) as wp,
            tc.tile_pool(name="xp", bufs=2) as xp,
            tc.tile_pool(name="op", bufs=4) as op,
            tc.tile_pool(name="pp", bufs=2, space="PSUM") as pp,
        ):
            slot_off = 0
            for j in range(EPC):
                cap = int(slot_caps[j])
                if cap == 0:
                    continue
                Tj = cap * 128
                # whole expert weight, k-tiled: (128, kt, 1408) bf16
                wt = wp.tile([128, KT, N], BF16, tag="w", name=f"w{j}")
                nc.sync.dma_start(
                    wt[:], w_d[j].rearrange("(kt p) n -> p kt n", p=128)
                )
                # slot's token columns of xT, k-tiled: (128, kt, Tj) bf16
                xt = xp.tile([128, KT, Tj], BF16, tag="x", name=f"x{j}")
                nc.sync.dma_start(
                    xt[:],
                    xt_d[:, slot_off : slot_off + Tj].rearrange(
                        "(kt p) t -> p kt t", p=128
                    ),
                )
                for m in range(cap):
                    ps = pp.tile([128, 3, 512], FP32, tag="ps", name=f"ps{j}_{m}")
                    for ni, (n0, nw) in enumerate(N_CHUNKS):
                        for k in range(KT):
                            nc.tensor.matmul(
                                ps[:, ni, 0:nw],
                                xt[:, k, m * 128 : (m + 1) * 128],
                                wt[:, k, n0 : n0 + nw],
                                start=(k == 0),
                                stop=(k == KT - 1),
                            )
                    ot = op.tile([128, N], FP32, tag="o", name=f"o{j}_{m}")
                    for ni, (n0, nw) in enumerate(N_CHUNKS):
                        nc.any.tensor_copy(ot[:, n0 : n0 + nw], ps[:, ni, 0:nw])
                    nc.sync.dma_start(
                        out_d[slot_off + m * 128 : slot_off + (m + 1) * 128, :],
                        ot[:],
                    )
                slot_off += Tj
    nc.compile()
    return nc


def _plan(bs):
    """Assign experts to (core, slot) and compute slot capacities."""
    order = np.argsort(-bs, kind="stable")  # experts sorted desc by size
    # slot j on core c handles expert order[8*j + c]
    assign = order.reshape(EPC, NCORES)
    caps = np.ceil(bs[assign].max(axis=1) / 128).astype(np.int64)
    return assign, caps


def kernel(x: np.ndarray, weight: np.ndarray, batch_sizes: np.ndarray) -> np.ndarray:
    global LAST_EXEC_NS, LAST_RESULTS
    x = np.asarray(x)
    weight = np.asarray(weight)
    bs = np.asarray(batch_sizes).astype(np.int64)
    assert x.shape == (S, M) and weight.shape == (E * M, N)

    assign, caps = _plan(bs)
    T_pad = 128 * int(caps.sum())
    key = tuple(caps.tolist())
    if key not in _program_cache:
        _program_cache[key] = _build_program(caps)
    nc = _program_cache[key]

    offs = np.concatenate([[0], np.cumsum(bs)])
    slot_offs = np.concatenate([[0], np.cumsum(caps * 128)])
    w3 = weight.reshape(E, M, N)

    xb = x.astype(ml_dtypes.bfloat16)
    in_maps = []
    for c in range(NCORES):
        xt_core = np.zeros((M, T_pad), dtype=ml_dtypes.bfloat16)
        w_core = np.empty((EPC, M, N), dtype=ml_dtypes.bfloat16)
        for j in range(EPC):
            e = int(assign[j, c])
            b = int(bs[e])
            xt_core[:, slot_offs[j] : slot_offs[j] + b] = xb[
                offs[e] : offs[e] + b
            ].T
            w_core[j] = w3[e]
        in_maps.append({"xt": xt_core, "w": w_core})

    trace = os.environ.get("BASS_KERNEL_TRACE", "1") != "0"
    try:
        res = run_bass_kernel_spmd(
            nc, in_maps, core_ids=list(range(NCORES)), trace=trace
        )
    except ModuleNotFoundError:
        # NTFF profiling hook unavailable in this image — run untraced.
        res = run_bass_kernel_spmd(
            nc, in_maps, core_ids=list(range(NCORES)), trace=False
        )
    LAST_RESULTS = res
    LAST_EXEC_NS = res.exec_time_ns

    out = np.empty((S, N), dtype=np.float32)
    for c in range(NCORES):
        core_out = res.results[c]["out"]
        for j in range(EPC):
            e = int(assign[j, c])
            b = int(bs[e])
            out[offs[e] : offs[e] + b] = core_out[
                slot_offs[j] : slot_offs[j] + b
            ]
    return out


# revision 3
# speedup vs baseline: 1.1922x; 1.1922x over previous
"""Grouped GEMM (MoE expert-parallel) Bass kernel for Trainium2.

Problem: x (16384, 2048) fp32, weight (128*2048, 1408) fp32, batch_sizes (128,)
int32 summing to 16384 (tokens sorted by expert).
out[rows_e] = x[rows_e] @ W[e] for each expert e.

Strategy (expert-parallel across 8 NeuronCores):
  - 16 experts per core. Experts are sorted by batch size (descending) and
    dealt round-robin into 16 "slots" x 8 cores, so slot j holds experts of
    similar size on every core. Slot j gets a static token capacity
    cap_j = max over cores of bs (rounded up to 16), making the compiled
    program identical (SPMD) across cores while keeping padding tiny.
  - Host pre-transposes/pads x to xT (2048, T_pad) bf16 per core (resident
    in SBUF for the whole kernel), reorders weights to (16, 2048, 1408)
    bf16 per core. On-chip: out = xT.T @ w via TensorE with K=2048
    contracted in 16 chunks of 128 accumulating in PSUM; m-tiles of up to
    128 tokens (the last tile of a slot may be <128 partitions).
  - Output (T_pad, 1408) bf16 per core; host strips padding, upcasts to
    fp32, and scatters rows back.

Self-contained: needs only numpy/ml_dtypes + the concourse package.
"""

import os

import numpy as np
import ml_dtypes

import concourse.bass as bass
import concourse.mybir as mybir
import concourse.tile as tile
from concourse import bacc
from concourse.bass_utils import run_bass_kernel_spmd

E = 128          # num experts
M = 2048         # in features (contraction)
N = 1408         # out features
S = 16384        # tokens
NCORES = 8
EPC = E // NCORES      # experts per core = 16
KT = M // 128          # contraction tiles = 16
N_CHUNKS = [(0, 512), (512, 512), (1024, 384)]  # psum-bank-sized N tiles

BF16 = mybir.dt.bfloat16
FP32 = mybir.dt.float32

_program_cache: dict = {}
LAST_EXEC_NS = None
LAST_RESULTS = None


def _build_program(slot_caps):
    """Compile the SPMD Bass program for the given per-slot token caps."""
    slot_caps = [int(c) for c in slot_caps]
    T_pad = sum(slot_caps)
    nc = bacc.Bacc(
        "TRN2", target_bir_lowering=False, debug=False, num_devices=NCORES
    )
    xt_d = nc.dram_tensor("xt", [M, T_pad], BF16, kind="ExternalInput").ap()
    w_d = nc.dram_tensor("w", [EPC, M, N], BF16, kind="ExternalInput").ap()
    out_d = nc.dram_tensor("out", [T_pad, N], BF16, kind="ExternalOutput").ap()

    with tile.TileContext(nc) as tc:
        with (
            tc.tile_pool(name="xp", bufs=1) as xp,
            tc.tile_pool(name="wp", bufs=2) as wp,
            tc.tile_pool(name="op", bufs=4) as op,
            tc.tile_pool(name="pp", bufs=2, space="PSUM") as pp,
        ):
            # whole xT resident in SBUF: (128, kt, T_pad) bf16
            xt = xp.tile([128, KT, T_pad], BF16, name="xall")
            nc.sync.dma_start(
                xt[:], xt_d.rearrange("(kt p) t -> p kt t", p=128)
            )
            slot_off = 0
            for j in range(EPC):
                cap = slot_caps[j]
                if cap == 0:
                    continue
                # whole expert weight, k-tiled: (128, kt, 1408) bf16
                wt = wp.tile([128, KT, N], BF16, tag="w", name=f"w{j}")
                nc.sync.dma_start(
                    wt[:], w_d[j].rearrange("(kt p) n -> p kt n", p=128)
                )
                m_off = 0
                while m_off < cap:
                    mr = min(128, cap - m_off)  # rows in this m-tile
                    t0 = slot_off + m_off
                    ps = pp.tile([128, 3, 512], FP32, tag="ps", name=f"ps{j}_{m_off}")
                    for ni, (n0, nw) in enumerate(N_CHUNKS):
                        for k in range(KT):
                            nc.tensor.matmul(
                                ps[0:mr, ni, 0:nw],
                                xt[:, k, t0 : t0 + mr],
                                wt[:, k, n0 : n0 + nw],
                                start=(k == 0),
                                stop=(k == KT - 1),
                            )
                    ot = op.tile([128, N], BF16, tag="o", name=f"o{j}_{m_off}")
                    for ni, (n0, nw) in enumerate(N_CHUNKS):
                        nc.any.tensor_copy(ot[0:mr, n0 : n0 + nw], ps[0:mr, ni, 0:nw])
                    nc.sync.dma_start(out_d[t0 : t0 + mr, :], ot[0:mr, :])
                    m_off += mr
                slot_off += cap
    nc.compile()
    return nc


def _plan(bs):
    """Assign experts to (core, slot) and compute slot capacities."""
    order = np.argsort(-bs, kind="stable")  # experts sorted desc by size
    # slot j on core c handles expert order[8*j + c]
    assign = order.reshape(EPC, NCORES)
    caps = bs[assign].max(axis=1)
    caps = ((caps + 15) // 16) * 16  # round to 16 for tidy strides
    return assign, caps.astype(np.int64)


def kernel(x: np.ndarray, weight: np.ndarray, batch_sizes: np.ndarray) -> np.ndarray:
    global LAST_EXEC_NS, LAST_RESULTS
    x = np.asarray(x)
    weight = np.asarray(weight)
    bs = np.asarray(batch_sizes).astype(np.int64)
    assert x.shape == (S, M) and weight.shape == (E * M, N)

    assign, caps = _plan(bs)
    T_pad = int(caps.sum())
    key = tuple(caps.tolist())
    if key not in _program_cache:
        _program_cache[key] = _build_program(caps)
    nc = _program_cache[key]

    offs = np.concatenate([[0], np.cumsum(bs)])
    slot_offs = np.concatenate([[0], np.cumsum(caps)])
    w3 = weight.reshape(E, M, N)

    xb = x.astype(ml_dtypes.bfloat16)
    in_maps = []
    for c in range(NCORES):
        xt_core = np.zeros((M, T_pad), dtype=ml_dtypes.bfloat16)
        w_core = np.empty((EPC, M, N), dtype=ml_dtypes.bfloat16)
        for j in range(EPC):
            e = int(assign[j, c])
            b = int(bs[e])
            xt_core[:, slot_offs[j] : slot_offs[j] + b] = xb[
                offs[e] : offs[e] + b
            ].T
            w_core[j] = w3[e]
        in_maps.append({"xt": xt_core, "w": w_core})

    trace = os.environ.get("BASS_KERNEL_TRACE", "1") != "0"
    try:
        res = run_bass_kernel_spmd(
            nc, in_maps, core_ids=list(range(NCORES)), trace=trace
        )
    except ModuleNotFoundError:
        # NTFF profiling hook unavailable in this image — run untraced.
        res = run_bass_kernel_spmd(
            nc, in_maps, core_ids=list(range(NCORES)), trace=False
        )
    LAST_RESULTS = res
    LAST_EXEC_NS = res.exec_time_ns

    out = np.empty((S, N), dtype=np.float32)
    for c in range(NCORES):
        core_out = res.results[c]["out"]
        for j in range(EPC):
            e = int(assign[j, c])
            b = int(bs[e])
            out[offs[e] : offs[e] + b] = core_out[
                slot_offs[j] : slot_offs[j] + b
            ].astype(np.float32)
    return out
